# revision 4
# baseline (speedup 1.0000x reference)
"""DLinear (causal-window decomposition + dual Linear) as a single fused matmul
on 8 Trainium2 NeuronCores.

Algebra: with A the [T,T] causal-window-mean operator (banded, window=25),
    trend        = x @ A^T
    out          = trend @ Tw^T + (x - trend) @ Sw^T + (tb + sb)
                 = x @ (Sw + (Tw - Sw) @ A)^T + (tb + sb)
A is banded with 25 nonzeros per row, so (Tw-Sw)@A is computed on the host in
O(T^2) via a reversed windowed column-sum.  The device then runs one
[2048,721] x [721,720] matmul per core (bias folded in as an extra
contraction row against a ones column appended to x).

Matmuls run in float32r (full-rate single-pass fp32 mode, ~1.6e-4 rel err).
x tiles are transposed on the PE (contraction dim must sit on partitions).
"""

import numpy as np

import concourse.bacc as bacc
import concourse.mybir as mybir
from concourse import tile, masks
from concourse.bass_utils import run_bass_kernel_spmd

WINDOW = 25
B, NPTS, T = 32, 512, 720
U = T                     # output features
N_CORES = 8
M_TOT = B * NPTS          # 16384 rows
M_LOC = M_TOT // N_CORES  # 2048 rows per core
P = 128                   # partitions
M_TILES = M_LOC // P      # 16
KE = T + 1                # contraction incl. bias row
K_CHUNKS = [(k * P, min(P, KE - k * P)) for k in range((KE + P - 1) // P)]
N_CHUNKS = [(0, 360), (360, 360)]  # PSUM-bank-sized slices of U, each >=256
XW = T + 8                # x tile free width: 720 data + col 720 = ones + pad

_F32 = mybir.dt.float32
_F32R = mybir.dt.float32r


def _build_nc():
    nc = bacc.Bacc("TRN2", target_bir_lowering=False, debug=False,
                   num_devices=N_CORES)
    xs = nc.dram_tensor("xs", [M_LOC, T], _F32, kind="ExternalInput").ap()
    wt = nc.dram_tensor("wt", [KE, U], _F32, kind="ExternalInput").ap()
    out = nc.dram_tensor("out", [M_LOC, U], _F32, kind="ExternalOutput").ap()

    with tile.TileContext(nc) as tc:
        with tc.tile_pool(name="wpool", bufs=1) as wpool, \
             tc.tile_pool(name="xpool", bufs=3) as xpool, \
             tc.tile_pool(name="xtpool", bufs=3) as xtpool, \
             tc.tile_pool(name="opool", bufs=3) as opool, \
             tc.tile_pool(name="tpp", bufs=2, space="PSUM") as tpp, \
             tc.tile_pool(name="accp", bufs=4, space="PSUM") as accp:

            ident = wpool.tile([P, P], _F32)
            masks.make_identity(nc, ident[:])

            # Weights: DMA in fp32, round once to f32r on DVE.
            w_tiles = []
            for k, (k0, kc) in enumerate(K_CHUNKS):
                w_raw = wpool.tile([P, U], _F32, name=f"w_raw{k}", tag=f"w_raw{k}")
                nc.sync.dma_start(w_raw[:kc, :], wt[k0:k0 + kc, :])
                w_r = wpool.tile([P, U], _F32R, name=f"w_r{k}", tag=f"w_r{k}")
                nc.vector.tensor_copy(w_r[:kc, :], w_raw[:kc, :])
                w_tiles.append(w_r)

            for m in range(M_TILES):
                xt = xpool.tile([P, XW], _F32, name="xt")
                nc.sync.dma_start(xt[:, 0:T], xs[m * P:(m + 1) * P, :])
                nc.gpsimd.memset(xt[:, T:T + 1], 1.0)  # bias ones column

                # Transpose [m,t] -> [t,m] on PE, packing 4+2 chunks per bank.
                ps_a = tpp.tile([P, 512], _F32, name="ps_a", tag="ps_a")
                ps_b = tpp.tile([P, 256], _F32, name="ps_b", tag="ps_b")
                for k, (k0, kc) in enumerate(K_CHUNKS):
                    dst = ps_a if k < 4 else ps_b
                    col = (k % 4) * P if k < 4 else (k - 4) * P
                    nc.tensor.transpose(
                        dst[0:kc, col:col + P], xt[:, k0:k0 + kc], ident[:])
                xT = xtpool.tile([P, 6 * P], _F32R, name="xT")
                nc.vector.tensor_copy(xT[:, 0:512], ps_a[:])
                nc.vector.tensor_copy(xT[:, 512:768], ps_b[:])

                ot = opool.tile([P, U], _F32, name="ot")
                for n, (n0, nw) in enumerate(N_CHUNKS):
                    acc = accp.tile([P, 512], _F32, name="acc", tag="acc")
                    for k, (k0, kc) in enumerate(K_CHUNKS):
                        nc.tensor.matmul(
                            acc[:, 0:nw],
                            xT[0:kc, k * P:k * P + P],
                            w_tiles[k][0:kc, n0:n0 + nw],
                            start=(k == 0), stop=(k == len(K_CHUNKS) - 1))
                    nc.scalar.copy(ot[:, n0:n0 + nw], acc[:, 0:nw])
                nc.sync.dma_start(out[m * P:(m + 1) * P, :], ot[:])

    nc.compile()
    return nc


def _fold_weights(trend_w, seasonal_w, trend_b, seasonal_b):
    """W = seasonal_w + (trend_w - seasonal_w) @ A, computed via the banded
    structure of A; returns [KE, U] = [W^T; b] ready for the device."""
    counts = np.minimum(np.arange(T) + 1, WINDOW).astype(np.float64)
    G = (trend_w.astype(np.float64) - seasonal_w.astype(np.float64)) / counts[None, :]
    M = np.zeros_like(G)
    for d in range(WINDOW):
        M[:, :T - d] += G[:, d:]
    W = seasonal_w.astype(np.float64) + M
    b = trend_b.astype(np.float64) + seasonal_b.astype(np.float64)
    wt_ext = np.empty((KE, U), np.float32)
    wt_ext[:T, :] = W.T.astype(np.float32)
    wt_ext[T, :] = b.astype(np.float32)
    return wt_ext


_NC_CACHE = {}
RUN_KWARGS = {}   # test harness may set {"trace": True}
LAST_RESULTS = None


def kernel(x, trend_w, trend_b, seasonal_w, seasonal_b):
    global LAST_RESULTS
    wt_ext = _fold_weights(trend_w, seasonal_w, trend_b, seasonal_b)
    xf = np.ascontiguousarray(np.asarray(x, dtype=np.float32)).reshape(M_TOT, T)

    if "nc" not in _NC_CACHE:
        _NC_CACHE["nc"] = _build_nc()
    nc = _NC_CACHE["nc"]

    in_maps = [
        {"xs": xf[i * M_LOC:(i + 1) * M_LOC], "wt": wt_ext}
        for i in range(N_CORES)
    ]
    res = run_bass_kernel_spmd(nc, in_maps, core_ids=list(range(N_CORES)),
                               **RUN_KWARGS)
    LAST_RESULTS = res
    out = np.concatenate([r["out"] for r in res.results], axis=0)
    return out.reshape(B, NPTS, U)


# revision 5
# speedup vs baseline: 1.0798x; 1.0798x over previous
"""DLinear (causal-window decomposition + dual Linear) as a single fused matmul
on 8 Trainium2 NeuronCores.

Algebra: with A the [T,T] causal-window-mean operator (banded, window=25),
    trend = x @ A^T
    out   = trend @ Tw^T + (x - trend) @ Sw^T + (tb + sb)
          = x @ (Sw + (Tw - Sw) @ A)^T + (tb + sb)
A is banded (25 nonzeros/row), so (Tw-Sw)@A folds on the host in O(T^2) via a
reversed windowed column-sum.  The device then runs one [2048,721] x [721,720]
matmul per core; the bias rides as an extra contraction row against a ones row
appended to x^T.

The per-core x shard is shipped pre-transposed ([721, 2048], contraction dim
on partitions) so the device does no PE transposes at all.  Matmuls run in
float32r (full-rate single-pass fp32 mode, ~1.5e-4 rel err).  Weights and x
are declared float32r in DRAM so no on-device rounding pass is needed.
"""

import numpy as np

import concourse.bacc as bacc
import concourse.mybir as mybir
from concourse import tile
from concourse.bass_utils import run_bass_kernel_spmd

WINDOW = 25
B, NPTS, T = 32, 512, 720
U = T                     # output features
N_CORES = 8
M_TOT = B * NPTS          # 16384 rows
M_LOC = M_TOT // N_CORES  # 2048 rows per core
P = 128                   # partitions
M_TILES = M_LOC // P      # 16
KE = T + 1                # contraction incl. bias row
K_CHUNKS = [(k * P, min(P, KE - k * P)) for k in range((KE + P - 1) // P)]
NK = len(K_CHUNKS)        # 6
N_CHUNKS = [(0, 360), (360, 360)]  # PSUM-bank-sized slices of U, each >=256
XG = 512                  # x column-group width for pipelined loading
N_GROUPS = M_LOC // XG    # 4

_F32 = mybir.dt.float32
_F32R = mybir.dt.float32r


def _build_nc():
    nc = bacc.Bacc("TRN2", target_bir_lowering=False, debug=False,
                   num_devices=N_CORES)
    xt_d = nc.dram_tensor("xt", [KE, M_LOC], _F32R, kind="ExternalInput").ap()
    wt_d = nc.dram_tensor("wt", [KE, U], _F32R, kind="ExternalInput").ap()
    out_d = nc.dram_tensor("out", [M_LOC, U], _F32, kind="ExternalOutput").ap()

    with tile.TileContext(nc) as tc:
        with tc.tile_pool(name="wpool", bufs=1) as wpool, \
             tc.tile_pool(name="xpool", bufs=1) as xpool, \
             tc.tile_pool(name="opool", bufs=3) as opool, \
             tc.tile_pool(name="accp", bufs=4, space="PSUM") as accp:

            # Weights resident in SBUF, loaded in n-halves on the SWDGE queue
            # so the first-matmul gate is [all w n0-halves + x group 0].
            w_tiles = [wpool.tile([P, U], _F32R, name=f"w{k}", tag=f"w{k}")
                       for k in range(NK)]
            for k, (k0, kc) in enumerate(K_CHUNKS):
                nc.gpsimd.dma_start(w_tiles[k][:kc, 0:360], wt_d[k0:k0 + kc, 0:360])

            # x^T chunks fully resident; column-group pipelined loads (HWDGE).
            xT = [xpool.tile([P, M_LOC], _F32R, name=f"x{k}", tag=f"x{k}")
                  for k in range(NK)]
            for g in range(N_GROUPS):
                if g == 1:
                    for k, (k0, kc) in enumerate(K_CHUNKS):
                        nc.gpsimd.dma_start(w_tiles[k][:kc, 360:720],
                                            wt_d[k0:k0 + kc, 360:720])
                for k, (k0, kc) in enumerate(K_CHUNKS):
                    nc.sync.dma_start(
                        xT[k][:kc, g * XG:(g + 1) * XG],
                        xt_d[k0:k0 + kc, g * XG:(g + 1) * XG])

            for mp in range(M_TILES // 2):      # m-tile pairs share one out DMA
                ot = opool.tile([P, 2 * U], _F32, name="ot")
                for half in range(2):
                    m = 2 * mp + half
                    for n, (n0, nw) in enumerate(N_CHUNKS):
                        acc = accp.tile([P, 512], _F32, name="acc", tag="acc")
                        for k, (k0, kc) in enumerate(K_CHUNKS):
                            nc.tensor.matmul(
                                acc[:, 0:nw],
                                xT[k][0:kc, m * P:(m + 1) * P],
                                w_tiles[k][0:kc, n0:n0 + nw],
                                start=(k == 0), stop=(k == NK - 1))
                        nc.scalar.copy(ot[:, half * U + n0:half * U + n0 + nw],
                                       acc[:, 0:nw])
                nc.scalar.dma_start(
                    out_d[2 * mp * P:(2 * mp + 2) * P, :].rearrange(
                        "(a p) u -> p a u", p=P),
                    ot[:].rearrange("p (a u) -> p a u", a=2))

    nc.compile()
    return nc


def _fold_weights(trend_w, seasonal_w, trend_b, seasonal_b):
    """W = seasonal_w + (trend_w - seasonal_w) @ A via the banded structure of
    A; returns [KE, U] = [W^T; b] ready for the device."""
    counts = np.minimum(np.arange(T) + 1, WINDOW).astype(np.float64)
    G = (trend_w.astype(np.float64) - seasonal_w.astype(np.float64)) / counts[None, :]
    M = np.zeros_like(G)
    for d in range(WINDOW):
        M[:, :T - d] += G[:, d:]
    W = seasonal_w.astype(np.float64) + M
    b = trend_b.astype(np.float64) + seasonal_b.astype(np.float64)
    wt_ext = np.empty((KE, U), np.float32)
    wt_ext[:T, :] = W.T.astype(np.float32)
    wt_ext[T, :] = b.astype(np.float32)
    return wt_ext


_NC_CACHE = {}
RUN_KWARGS = {}   # test harness may set {"trace": True}
LAST_RESULTS = None


def kernel(x, trend_w, trend_b, seasonal_w, seasonal_b):
    global LAST_RESULTS
    wt_ext = _fold_weights(trend_w, seasonal_w, trend_b, seasonal_b)

    # Pre-transposed, ones-row-extended per-core shards: [8, 721, 2048].
    x2d = np.asarray(x, dtype=np.float32).reshape(M_TOT, T)
    xt_all = np.empty((KE, M_TOT), np.float32)
    xt_all[:T] = x2d.T
    xt_all[T] = 1.0
    xt_cores = np.ascontiguousarray(
        xt_all.reshape(KE, N_CORES, M_LOC).transpose(1, 0, 2))

    if "nc" not in _NC_CACHE:
        _NC_CACHE["nc"] = _build_nc()
    nc = _NC_CACHE["nc"]

    in_maps = [{"xt": xt_cores[i], "wt": wt_ext} for i in range(N_CORES)]
    res = run_bass_kernel_spmd(nc, in_maps, core_ids=list(range(N_CORES)),
                               **RUN_KWARGS)
    LAST_RESULTS = res
    out = np.concatenate([r["out"] for r in res.results], axis=0)
    return out.reshape(B, NPTS, U)


# revision 7
# speedup vs baseline: 1.1923x; 1.1042x over previous
"""DLinear (causal-window decomposition + dual Linear) as a single fused matmul
on 8 Trainium2 NeuronCores.

Algebra: with A the [T,T] causal-window-mean operator (banded, window=25),
    trend = x @ A^T
    out   = trend @ Tw^T + (x - trend) @ Sw^T + (tb + sb)
          = x @ (Sw + (Tw - Sw) @ A)^T + (tb + sb)
A is banded (25 nonzeros/row), so (Tw-Sw)@A folds on the host in O(T^2) via a
reversed windowed column-sum.  The device then runs one [2048,721] x [721,720]
matmul per core; the bias rides as an extra contraction row against a ones row
appended to x^T.

The per-core x shard is shipped pre-transposed ([721, 2048], contraction dim
on partitions) so the device does no PE transposes at all.  Matmuls run in
float32r (full-rate single-pass fp32 mode, ~1.5e-4 rel err).  Weights and x
are declared float32r in DRAM so no on-device rounding pass is needed.
"""

import numpy as np

import concourse.bacc as bacc
import concourse.mybir as mybir
from concourse import tile
from concourse.bass_utils import run_bass_kernel_spmd

WINDOW = 25
B, NPTS, T = 32, 512, 720
U = T                     # output features
N_CORES = 8
M_TOT = B * NPTS          # 16384 rows
M_LOC = M_TOT // N_CORES  # 2048 rows per core
P = 128                   # partitions
M_TILES = M_LOC // P      # 16
KE = T + 1                # contraction incl. bias row
K_CHUNKS = [(k * P, min(P, KE - k * P)) for k in range((KE + P - 1) // P)]
NK = len(K_CHUNKS)        # 6
N_CHUNKS = [(0, 360), (360, 360)]  # PSUM-bank-sized slices of U, each >=256
# x column-group widths for pipelined loading; first small to start PE early
GROUPS = [(0, 256), (256, 512), (768, 512), (1280, 512), (1792, 256)]

_F32 = mybir.dt.float32
_F32R = mybir.dt.float32r
_BF16 = mybir.dt.bfloat16
N_WARMUP = 10             # junk bf16 matmuls to lift the PE HAM clock-gate


def _build_nc():
    nc = bacc.Bacc("TRN2", target_bir_lowering=False, debug=False,
                   num_devices=N_CORES)
    xt_d = nc.dram_tensor("xt", [KE, M_LOC], _F32R, kind="ExternalInput").ap()
    wt_d = nc.dram_tensor("wt", [KE, U], _F32R, kind="ExternalInput").ap()
    out_d = nc.dram_tensor("out", [M_LOC, U], _F32, kind="ExternalOutput").ap()

    with tile.TileContext(nc) as tc:
        with tc.tile_pool(name="wpool", bufs=1) as wpool, \
             tc.tile_pool(name="xpool", bufs=1) as xpool, \
             tc.tile_pool(name="opool", bufs=4) as opool, \
             tc.tile_pool(name="wup", bufs=1, space="PSUM") as wup, \
             tc.tile_pool(name="accp", bufs=6, space="PSUM") as accp:

            # HAM warm-up: junk bf16 matmuls keep the PE busy while the first
            # DMAs land, so real matmuls start at the 2.4 GHz clock.
            scr = wpool.tile([P, 512], _BF16, name="scr", tag="scr")
            nc.gpsimd.memset(scr[:], 0.0)
            ps_scr = wup.tile([P, 512], _F32, name="ps_scr", tag="ps_scr")
            for _ in range(N_WARMUP):
                nc.tensor.matmul(ps_scr[:], scr[:, 0:P], scr[:],
                                 start=True, stop=True)

            # Weights resident in SBUF, loaded in n-halves on the SWDGE queue
            # so the first-matmul gate is [all w n0-halves + x group 0].
            w_tiles = [wpool.tile([P, U], _F32R, name=f"w{k}", tag=f"w{k}")
                       for k in range(NK)]
            for k, (k0, kc) in enumerate(K_CHUNKS):
                nc.gpsimd.dma_start(w_tiles[k][:kc, 0:360], wt_d[k0:k0 + kc, 0:360])

            # x^T chunks fully resident; column-group pipelined loads (HWDGE).
            xT = [xpool.tile([P, M_LOC], _F32R, name=f"x{k}", tag=f"x{k}")
                  for k in range(NK)]
            for g, (c0, cw) in enumerate(GROUPS):
                if g == 1:
                    for k, (k0, kc) in enumerate(K_CHUNKS):
                        nc.gpsimd.dma_start(w_tiles[k][:kc, 360:720],
                                            wt_d[k0:k0 + kc, 360:720])
                for k, (k0, kc) in enumerate(K_CHUNKS):
                    nc.sync.dma_start(xT[k][:kc, c0:c0 + cw],
                                      xt_d[k0:k0 + kc, c0:c0 + cw])

            # (m, n) schedule: n-major inside group 0 so the first matmuls
            # need only the n0 half of the weights.
            plan = []
            for g, (c0, cw) in enumerate(GROUPS):
                ms = range(c0 // P, (c0 + cw) // P)
                if g == 0:
                    plan += [(m, 0) for m in ms] + [(m, 1) for m in ms]
                else:
                    plan += [(m, n) for m in ms for n in (0, 1)]

            ot_tiles = {}
            done = {}
            for m, n in plan:
                if m not in ot_tiles:
                    ot_tiles[m] = opool.tile([P, U], _F32, name="ot")
                n0, nw = N_CHUNKS[n]
                acc = accp.tile([P, 512], _F32, name="acc", tag="acc")
                for k, (k0, kc) in enumerate(K_CHUNKS):
                    nc.tensor.matmul(
                        acc[:, 0:nw],
                        xT[k][0:kc, m * P:(m + 1) * P],
                        w_tiles[k][0:kc, n0:n0 + nw],
                        start=(k == 0), stop=(k == NK - 1))
                nc.scalar.copy(ot_tiles[m][:, n0:n0 + nw], acc[:, 0:nw])
                done[m] = done.get(m, 0) + 1
                if done[m] == 2:
                    nc.scalar.dma_start(out_d[m * P:(m + 1) * P, :],
                                        ot_tiles.pop(m)[:])

    nc.compile()
    return nc


def _fold_weights(trend_w, seasonal_w, trend_b, seasonal_b):
    """W = seasonal_w + (trend_w - seasonal_w) @ A via the banded structure of
    A; returns [KE, U] = [W^T; b] ready for the device."""
    counts = np.minimum(np.arange(T) + 1, WINDOW).astype(np.float64)
    G = (trend_w.astype(np.float64) - seasonal_w.astype(np.float64)) / counts[None, :]
    M = np.zeros_like(G)
    for d in range(WINDOW):
        M[:, :T - d] += G[:, d:]
    W = seasonal_w.astype(np.float64) + M
    b = trend_b.astype(np.float64) + seasonal_b.astype(np.float64)
    wt_ext = np.empty((KE, U), np.float32)
    wt_ext[:T, :] = W.T.astype(np.float32)
    wt_ext[T, :] = b.astype(np.float32)
    return wt_ext


_NC_CACHE = {}
RUN_KWARGS = {}   # test harness may set {"trace": True}
LAST_RESULTS = None


def kernel(x, trend_w, trend_b, seasonal_w, seasonal_b):
    global LAST_RESULTS
    wt_ext = _fold_weights(trend_w, seasonal_w, trend_b, seasonal_b)

    # Pre-transposed, ones-row-extended per-core shards: [8, 721, 2048].
    x2d = np.asarray(x, dtype=np.float32).reshape(M_TOT, T)
    xt_all = np.empty((KE, M_TOT), np.float32)
    xt_all[:T] = x2d.T
    xt_all[T] = 1.0
    xt_cores = np.ascontiguousarray(
        xt_all.reshape(KE, N_CORES, M_LOC).transpose(1, 0, 2))

    if "nc" not in _NC_CACHE:
        _NC_CACHE["nc"] = _build_nc()
    nc = _NC_CACHE["nc"]

    in_maps = [{"xt": xt_cores[i], "wt": wt_ext} for i in range(N_CORES)]
    res = run_bass_kernel_spmd(nc, in_maps, core_ids=list(range(N_CORES)),
                               **RUN_KWARGS)
    LAST_RESULTS = res
    out = np.concatenate([r["out"] for r in res.results], axis=0)
    return out.reshape(B, NPTS, U)


# revision 8
# speedup vs baseline: 1.4200x; 1.1910x over previous
"""DLinear (causal-window decomposition + dual Linear) as a single fused matmul
on 8 Trainium2 NeuronCores.

Algebra: with A the [T,T] causal-window-mean operator (banded, window=25),
    trend = x @ A^T
    out   = trend @ Tw^T + (x - trend) @ Sw^T + (tb + sb)
          = x @ (Sw + (Tw - Sw) @ A)^T + (tb + sb)
A is banded (25 nonzeros/row), so (Tw-Sw)@A folds on the host in O(T^2) via a
reversed windowed column-sum.  The device then runs one [2048,721] x [721,720]
matmul per core; the bias rides as an extra contraction row against a ones row
appended to x^T.

The per-core x shard is shipped pre-transposed ([721, 2048], contraction dim
on partitions) so the device does no PE transposes.  Data moves as fp16
(x, W, out) with fp32 PSUM accumulation: fp16xfp16 products are exact in
fp32, end-to-end error ~5e-4 of output scale while halving DMA traffic,
which is the binding resource.  Output is upcast to fp32 on the host.
"""

import numpy as np

import concourse.bacc as bacc
import concourse.mybir as mybir
from concourse import tile
from concourse.bass_utils import run_bass_kernel_spmd

WINDOW = 25
B, NPTS, T = 32, 512, 720
U = T                     # output features
N_CORES = 8
M_TOT = B * NPTS          # 16384 rows
M_LOC = M_TOT // N_CORES  # 2048 rows per core
P = 128                   # partitions
M_TILES = M_LOC // P      # 16
KE = T + 1                # contraction incl. bias row
K_CHUNKS = [(k * P, min(P, KE - k * P)) for k in range((KE + P - 1) // P)]
NK = len(K_CHUNKS)        # 6
N_CHUNKS = [(0, 360), (360, 360)]  # PSUM-bank-sized slices of U
# x column-group widths for pipelined loading; first small to start PE early
GROUPS = [(0, 256), (256, 512), (768, 512), (1280, 512), (1792, 256)]

_F32 = mybir.dt.float32
_F16 = mybir.dt.float16
N_WARMUP = 8              # junk matmuls to lift the PE HAM clock-gate


def _build_nc():
    nc = bacc.Bacc("TRN2", target_bir_lowering=False, debug=False,
                   num_devices=N_CORES)
    xt_d = nc.dram_tensor("xt", [KE, M_LOC], _F16, kind="ExternalInput").ap()
    wt_d = nc.dram_tensor("wt", [KE, U], _F16, kind="ExternalInput").ap()
    out_d = nc.dram_tensor("out", [M_LOC, U], _F16, kind="ExternalOutput").ap()

    with tile.TileContext(nc) as tc:
        with tc.tile_pool(name="wpool", bufs=1) as wpool, \
             tc.tile_pool(name="xpool", bufs=1) as xpool, \
             tc.tile_pool(name="opool", bufs=4) as opool, \
             tc.tile_pool(name="wup", bufs=1, space="PSUM") as wup, \
             tc.tile_pool(name="accp", bufs=6, space="PSUM") as accp:

            # HAM warm-up: junk matmuls keep the PE busy while the first
            # DMAs land, so real matmuls start at the 2.4 GHz clock.
            scr = wpool.tile([P, 512], _F16, name="scr", tag="scr")
            nc.gpsimd.memset(scr[:], 0.0)
            ps_scr = wup.tile([P, 512], _F32, name="ps_scr", tag="ps_scr")
            for _ in range(N_WARMUP):
                nc.tensor.matmul(ps_scr[:], scr[:, 0:P], scr[:],
                                 start=True, stop=True)

            # Weights resident in SBUF, loaded in n-halves on the SWDGE queue
            # so the first-matmul gate is [all w n0-halves + x group 0].
            w_tiles = [wpool.tile([P, U], _F16, name=f"w{k}", tag=f"w{k}")
                       for k in range(NK)]
            for k, (k0, kc) in enumerate(K_CHUNKS):
                nc.gpsimd.dma_start(w_tiles[k][:kc, 0:360], wt_d[k0:k0 + kc, 0:360])

            # x^T chunks fully resident; column-group pipelined loads (HWDGE).
            xT = [xpool.tile([P, M_LOC], _F16, name=f"x{k}", tag=f"x{k}")
                  for k in range(NK)]
            for g, (c0, cw) in enumerate(GROUPS):
                if g == 1:
                    for k, (k0, kc) in enumerate(K_CHUNKS):
                        nc.gpsimd.dma_start(w_tiles[k][:kc, 360:720],
                                            wt_d[k0:k0 + kc, 360:720])
                for k, (k0, kc) in enumerate(K_CHUNKS):
                    nc.sync.dma_start(xT[k][:kc, c0:c0 + cw],
                                      xt_d[k0:k0 + kc, c0:c0 + cw])

            # (m, n) schedule: n-major inside group 0 so the first matmuls
            # need only the n0 half of the weights.
            plan = []
            for g, (c0, cw) in enumerate(GROUPS):
                ms = range(c0 // P, (c0 + cw) // P)
                if g == 0:
                    plan += [(m, 0) for m in ms] + [(m, 1) for m in ms]
                else:
                    plan += [(m, n) for m in ms for n in (0, 1)]

            ot_tiles = {}
            done = {}
            for m, n in plan:
                if m not in ot_tiles:
                    ot_tiles[m] = opool.tile([P, U], _F16, name="ot")
                n0, nw = N_CHUNKS[n]
                acc = accp.tile([P, 512], _F32, name="acc", tag="acc")
                for k, (k0, kc) in enumerate(K_CHUNKS):
                    nc.tensor.matmul(
                        acc[:, 0:nw],
                        xT[k][0:kc, m * P:(m + 1) * P],
                        w_tiles[k][0:kc, n0:n0 + nw],
                        start=(k == 0), stop=(k == NK - 1))
                nc.scalar.copy(ot_tiles[m][:, n0:n0 + nw], acc[:, 0:nw])
                done[m] = done.get(m, 0) + 1
                if done[m] == 2:
                    nc.scalar.dma_start(out_d[m * P:(m + 1) * P, :],
                                        ot_tiles.pop(m)[:])

    nc.compile()
    return nc


def _fold_weights(trend_w, seasonal_w, trend_b, seasonal_b):
    """W = seasonal_w + (trend_w - seasonal_w) @ A via the banded structure of
    A; returns [KE, U] = [W^T; b] ready for the device."""
    counts = np.minimum(np.arange(T) + 1, WINDOW).astype(np.float64)
    G = (trend_w.astype(np.float64) - seasonal_w.astype(np.float64)) / counts[None, :]
    M = np.zeros_like(G)
    for d in range(WINDOW):
        M[:, :T - d] += G[:, d:]
    W = seasonal_w.astype(np.float64) + M
    b = trend_b.astype(np.float64) + seasonal_b.astype(np.float64)
    wt_ext = np.empty((KE, U), np.float32)
    wt_ext[:T, :] = W.T.astype(np.float32)
    wt_ext[T, :] = b.astype(np.float32)
    return wt_ext


_NC_CACHE = {}
RUN_KWARGS = {}   # test harness may set {"trace": True}
LAST_RESULTS = None


def kernel(x, trend_w, trend_b, seasonal_w, seasonal_b):
    global LAST_RESULTS
    wt_ext = _fold_weights(trend_w, seasonal_w, trend_b, seasonal_b)

    # Pre-transposed, ones-row-extended per-core fp16 shards: [8, 721, 2048].
    x2d = np.asarray(x, dtype=np.float32).reshape(M_TOT, T)
    xt_all = np.empty((KE, M_TOT), np.float16)
    xt_all[:T] = x2d.T.astype(np.float16)
    xt_all[T] = 1.0
    xt_cores = np.ascontiguousarray(
        xt_all.reshape(KE, N_CORES, M_LOC).transpose(1, 0, 2))

    if "nc" not in _NC_CACHE:
        _NC_CACHE["nc"] = _build_nc()
    nc = _NC_CACHE["nc"]

    wt16 = wt_ext.astype(np.float16)
    in_maps = [{"xt": xt_cores[i], "wt": wt16} for i in range(N_CORES)]
    res = run_bass_kernel_spmd(nc, in_maps, core_ids=list(range(N_CORES)),
                               **RUN_KWARGS)
    LAST_RESULTS = res
    out = np.concatenate([r["out"] for r in res.results], axis=0)
    return out.astype(np.float32).reshape(B, NPTS, U)
